# revision 24
# baseline (speedup 1.0000x reference)
# Trainium2 Bass kernel for nn_BasicBlock (ShiftNet/AdderNet basic block).
#
# Reference computation (per full batch of 32 images):
#   y1 = conv3x3(x, quantize_pow2(w_shift1))          # power-of-two weights
#   z1 = -SAD3x3(y1, w_add1)                          # adder conv: -sum |patch - w|
#   a1 = relu(batchnorm_train(z1, g1, b1))            # batch stats over (N,H,W)
#   y2 = conv3x3(a1, quantize_pow2(w_shift2))
#   z2 = -SAD3x3(y2, w_add2)
#   out = relu(batchnorm_train(z2, g2, b2) + x)
#
# Strategy (8 NeuronCores, data-parallel over batch, 4 images/core):
#   - shift conv: 9 accumulating PE matmuls per output tile (im2col-free,
#     shifted reads of a zero-padded plane in SBUF).
#   - adder conv (dominant cost): for each (out-channel co, kernel-pos kk),
#     produce D = |y_shifted - w[co,:,kk]| as a [128, n_img*900] tile
#     (DVE tensor_scalar(subtract, abs_max) at 2x fp32, or ScalarE
#     activation(Abs, scale=-1, bias=w) -- split across both engines),
#     then reduce over (ci, kk) with PE matmuls whose stationary operand is a
#     ones-column one-hot matrix selecting PSUM row co; 9*128 matmuls
#     accumulate into each PSUM tile.
#   - batchnorm: per-core partial sums (via ScalarE accum_out during PSUM
#     evacuation) + a 1KB AllReduce across the 8 cores; scale/bias folded
#     (including the z = -S sign flip) into a single ScalarE
#     relu(scale*S + bias) with per-partition scale/bias.
import os
from contextlib import ExitStack

import numpy as np
import ml_dtypes

import concourse.bass as bass
import concourse.tile as tile
from concourse import bacc, mybir

F32 = mybir.dt.float32
BF16 = mybir.dt.bfloat16
AF = mybir.ActivationFunctionType
ALU = mybir.AluOpType

# Problem constants (hardcoded per spec nn_BasicBlock_21131239097114)
N_FULL = 32
C_FULL = 128
H = W = 28
KK = 9           # 3x3 kernel positions
PH = PW = 30     # padded plane
PLANE = PH * PW  # 900
L = H * W        # 784
NTILE = 392      # matmul free dim = half an image plane (<=512 fp32 PSUM bank)
EPS = 1e-5
THRESH = 0.005
N_CORES = 8
N_IMG = N_FULL // N_CORES

# kernel positions handled by ScalarE (vs VectorE) when producing |y - w|
ACT_KK = (2, 5, 8)


def shift_quant_np(w: np.ndarray) -> np.ndarray:
    """numpy mirror of reference.shift_quant (fp32 semantics)."""
    w = w.astype(np.float32)
    aw = np.abs(w)
    q = np.sign(w) * np.exp2(np.round(np.log2(np.maximum(aw, np.float32(1e-10)))))
    q = np.where(aw < np.float32(THRESH), np.float32(0.0), q).astype(np.float32)
    return q


def build_body(tc, out_ap, x_ap, wq_ap, wadd_ap, oh_ap, gb_ap,
               c: int, n_img: int, n_cores: int, dbg=None):
    nc = tc.nc
    PL = n_img * PLANE
    n_t = 2 * n_img                    # psum tiles per adder phase
    count = n_cores * n_img * L        # global batchnorm element count
    inv_cnt = 1.0 / float(count)

    with ExitStack() as ctx:
        sing = ctx.enter_context(tc.tile_pool(name="sing", bufs=1))
        dpool = ctx.enter_context(tc.tile_pool(name="dpool", bufs=6))
        sqpool = ctx.enter_context(tc.tile_pool(name="sqpool", bufs=2))
        dram = ctx.enter_context(tc.tile_pool(name="drampool", bufs=1, space="DRAM"))

        x_pad = sing.tile([c, PL + 64], F32, tag="x_pad")
        y_pad = sing.tile([c, PL + 64], F32, tag="y_pad")   # reused: y1 then y2
        a_pad = sing.tile([c, PL + 64], F32, tag="a_pad")
        S_sb = sing.tile([c, n_img, L], F32, tag="S_sb")    # reused: S1 then S2
        o_sb = sing.tile([c, n_img, L], F32, tag="o_sb")
        wq_sb = sing.tile([c, 2, KK, c], F32, tag="wq_sb")
        wadd_sb = sing.tile([c, 2, c, KK], F32, tag="wadd_sb")
        oh_sb = sing.tile([c, 4 * c], BF16, tag="oh_sb")
        allones = sing.tile([c, c], F32, tag="allones")
        gb_sb = sing.tile([c, 4], F32, tag="gb_sb")
        consts = sing.tile([c, 3], F32, tag="consts")       # [0, eps, 1]
        sums = sing.tile([c, 2 * n_t], F32, tag="sums")     # [sum S | sum S^2]
        stats = sing.tile([c, 2], F32, tag="stats")
        statsg = sing.tile([c, 2], F32, tag="statsg")
        bnw = sing.tile([c, 12], F32, tag="bnw")

        for t in (x_pad, y_pad, a_pad):
            nc.vector.memset(t[:, :], 0.0)
        nc.vector.memset(consts[:, 0:1], 0.0)
        nc.vector.memset(consts[:, 1:2], float(EPS))
        nc.vector.memset(consts[:, 2:3], 1.0)
        zero_c, eps_c, ones_c = consts[:, 0:1], consts[:, 1:2], consts[:, 2:3]
        nc.vector.memset(allones[:, :], 1.0)

        def pview(t):
            return t[:, :PL].rearrange("p (n ph pw) -> p n ph pw", ph=PH, pw=PW)

        xv = pview(x_pad)
        for n in range(n_img):
            nc.sync.dma_start(out=xv[:, n, 1:1 + H, 1:1 + W],
                              in_=x_ap[n].rearrange("c h w -> c h w"))
        nc.sync.dma_start(out=wq_sb[:, :, :, :],
                          in_=wq_ap.rearrange("l k i o -> i l k o"))
        nc.sync.dma_start(out=wadd_sb[:, :, :, :], in_=wadd_ap)
        nc.sync.dma_start(out=oh_sb[:, :], in_=oh_ap)
        nc.sync.dma_start(out=gb_sb[:, :], in_=gb_ap)

        def conv(layer: int, src_pad, dst_pad):
            srcv = pview(src_pad)
            dstv = pview(dst_pad)
            with tc.tile_pool(name=f"psc{layer}", bufs=2, space="PSUM") as pp:
                for n in range(n_img):
                    for hf in range(2):
                        h0 = hf * 14
                        ps = pp.tile([c, NTILE], F32, tag="cps")
                        for kk in range(KK):
                            dh, dw = divmod(kk, 3)
                            rhs = srcv[:, n, h0 + dh:h0 + dh + 14, dw:dw + W]
                            nc.tensor.matmul(ps[:, :], lhsT=wq_sb[:, layer, kk, :],
                                             rhs=rhs,
                                             start=(kk == 0), stop=(kk == KK - 1))
                        nc.scalar.activation(
                            out=dstv[:, n, 1 + h0:15 + h0, 1:1 + W],
                            in_=ps[:, :].rearrange("p (a b) -> p a b", a=14),
                            func=AF.Copy)

        def adder_and_stats(layer: int, src_pad):
            """S_sb[co,n,l] = S'[co,n,l] = sum|y - w| + const(co), computed as
            sum_{ci,kk} y  +  sum_{ci,kk} 2*relu(w - y); the per-channel const
            shift cancels in train-mode BN (shift invariance). Also accumulates
            per-core [sum S', sum S'^2] into stats."""
            srcv = pview(src_pad)
            with tc.tile_pool(name=f"psa{layer}", bufs=n_t, space="PSUM") as pa:
                Ts = [pa.tile([c, NTILE], F32, tag="aps", name=f"aps{layer}_{t}")
                      for t in range(n_t)]
                # SumY broadcast into every output row: 9 all-ones fp32 matmuls
                for t in range(n_t):
                    n, hf = divmod(t, 2)
                    h0 = hf * 14
                    for kk in range(KK):
                        dh, dw = divmod(kk, 3)
                        rhs = srcv[:, n, h0 + dh:h0 + dh + 14, dw:dw + W]
                        nc.tensor.matmul(Ts[t][:, :], lhsT=allones[:, :], rhs=rhs,
                                         start=(kk == 0), stop=False)
                for co in range(c):
                    for kk in range(KK):
                        dh, dw = divmod(kk, 3)
                        delta = dh * PW + dw
                        D = dpool.tile([c, PL], BF16, tag="D")
                        w_col = wadd_sb[:, layer, co, kk:kk + 1]
                        src = src_pad[:, delta:delta + PL]
                        if kk in ACT_KK:
                            # relu(w - y); reduced with the +2.0 one-hot
                            nc.scalar.activation(out=D[:, :], in_=src,
                                                 func=AF.Relu,
                                                 bias=w_col, scale=-1.0)
                            lhsT = oh_sb[:, 3 * c - co:4 * c - co]
                        else:
                            # min(y - w, 0) = -relu(w - y); the -2.0 one-hot
                            nc.vector.tensor_scalar(out=D[:, :], in0=src,
                                                    scalar1=w_col, scalar2=0.0,
                                                    op0=ALU.subtract,
                                                    op1=ALU.min)
                            lhsT = oh_sb[:, c - co:2 * c - co]
                        Dv = D[:, :].rearrange("p (n ph pw) -> p n ph pw",
                                               ph=PH, pw=PW)
                        for t in range(n_t):
                            n, hf = divmod(t, 2)
                            h0 = hf * 14
                            rhs = Dv[:, n, h0:h0 + 14, 0:W]
                            nc.tensor.matmul(
                                Ts[t][:, :], lhsT=lhsT, rhs=rhs,
                                start=False,
                                stop=(co == c - 1 and kk == KK - 1))
                # evacuate PSUM -> SBUF, accumulating BN partial sums for free
                for t in range(n_t):
                    n, hf = divmod(t, 2)
                    h0 = hf * 14
                    sv = S_sb[:, n, h0 * W:(h0 + 14) * W]
                    nc.scalar.activation(out=sv, in_=Ts[t][:, :], func=AF.Copy,
                                         accum_out=sums[:, t:t + 1])
                    sq = sqpool.tile([c, NTILE], F32, tag="sq")
                    nc.scalar.activation(out=sq[:, :], in_=Ts[t][:, :],
                                         func=AF.Square, bias=zero_c,
                                         accum_out=sums[:, n_t + t:n_t + t + 1])
            nc.vector.tensor_reduce(out=stats[:, 0:1], in_=sums[:, 0:n_t],
                                    axis=mybir.AxisListType.X, op=ALU.add)
            nc.vector.tensor_reduce(out=stats[:, 1:2], in_=sums[:, n_t:2 * n_t],
                                    axis=mybir.AxisListType.X, op=ALU.add)

        def bn_scales(layer: int):
            """AllReduce stats; return ([c,1] scale, [c,1] bias) APs such that
            bn_out = scale*S + bias  (includes the z = -S sign fold)."""
            cin = dram.tile([c, 2], F32, tag=f"cin{layer}")
            nc.gpsimd.dma_start(out=cin[:, :], in_=stats[:, :])
            if n_cores > 1:
                cout = dram.tile([c, 2], F32, tag=f"cout{layer}")
                nc.gpsimd.collective_compute(
                    "AllReduce", ALU.add,
                    replica_groups=[list(range(n_cores))],
                    ins=[cin.opt()], outs=[cout.opt()])
                nc.gpsimd.dma_start(out=statsg[:, :], in_=cout[:, :])
            else:
                nc.gpsimd.dma_start(out=statsg[:, :], in_=cin[:, :])

            def col(i):
                return bnw[:, i:i + 1]
            v = nc.vector
            v.tensor_scalar_mul(col(0), statsg[:, 0:1], inv_cnt)        # mean(S)
            v.tensor_scalar_mul(col(1), statsg[:, 1:2], inv_cnt)        # E[S^2]
            v.tensor_mul(col(2), col(0), col(0))                        # mean^2
            v.tensor_sub(col(3), col(1), col(2))                        # var
            nc.scalar.activation(out=col(4), in_=col(3), func=AF.Sqrt,
                                 bias=eps_c)                            # sqrt(var+eps)
            v.reciprocal(col(5), col(4))                                # r0 ~ rsqrt
            v.tensor_scalar_add(col(6), col(3), float(EPS))             # v = var+eps
            v.tensor_mul(col(7), col(5), col(5))                        # r0^2
            v.tensor_mul(col(7), col(7), col(6))                        # v*r0^2
            v.tensor_scalar(out=col(7), in0=col(7), scalar1=-0.5, scalar2=1.5,
                            op0=ALU.mult, op1=ALU.add)                  # 1.5-0.5*v*r0^2
            v.tensor_mul(col(5), col(5), col(7))                        # refined rsqrt
            g = gb_sb[:, 2 * layer:2 * layer + 1]
            b = gb_sb[:, 2 * layer + 1:2 * layer + 2]
            v.tensor_mul(col(8), g, col(5))                             # gamma*r
            v.tensor_scalar_mul(col(9), col(8), -1.0)                   # scale=-gamma*r
            v.tensor_mul(col(10), col(0), col(8))                       # mu*gamma*r
            v.tensor_add(col(10), col(10), b)                           # bias
            return col(9), col(10)

        # ---- layer 1 ----
        conv(0, x_pad, y_pad)
        if dbg is not None and "y1" in dbg:
            nc.sync.dma_start(out=dbg["y1"], in_=y_pad[:, :PL])
        adder_and_stats(0, y_pad)
        if dbg is not None and "S1" in dbg:
            nc.sync.dma_start(out=dbg["S1"], in_=S_sb[:, :, :])
        scale1, bias1 = bn_scales(0)
        av = pview(a_pad)[:, :, 1:1 + H, 1:1 + W]
        sve = S_sb[:, :, :].rearrange("p n (h w) -> p n h w", h=H)
        nc.scalar.activation(out=av, in_=sve, func=AF.Relu,
                             scale=scale1, bias=bias1)

        # ---- layer 2 ----
        conv(1, a_pad, y_pad)
        adder_and_stats(1, y_pad)
        scale2, bias2 = bn_scales(1)

        # out = relu(scale2*S2 + bias2 + x)
        nc.vector.tensor_scalar(out=o_sb[:, :, :], in0=S_sb[:, :, :],
                                scalar1=scale2, scalar2=bias2,
                                op0=ALU.mult, op1=ALU.add)
        ov = o_sb[:, :, :].rearrange("p n (h w) -> p n h w", h=H)
        nc.vector.tensor_add(ov, ov, xv[:, :, 1:1 + H, 1:1 + W])
        nc.scalar.activation(out=o_sb[:, :, :], in_=o_sb[:, :, :], func=AF.Relu,
                             bias=zero_c)
        nc.sync.dma_start(out=out_ap.rearrange("n c h w -> c n (h w)"),
                          in_=o_sb[:, :, :])


def prep_weights(w_shift1, w_add1, w_shift2, w_add2, bn1_gamma, bn1_beta,
                 bn2_gamma, bn2_beta, c: int):
    """Host-side packing. Returns dict of device input arrays (minus x)."""
    wq = np.zeros((2, KK, c, c), np.float32)
    for layer, w in ((0, w_shift1), (1, w_shift2)):
        q = shift_quant_np(np.asarray(w, np.float32))       # [co, ci, kh, kw]
        for kk in range(KK):
            kh, kw = divmod(kk, 3)
            wq[layer, kk] = q[:, :, kh, kw].T                # [ci, co]
    wadd = np.zeros((c, 2, c, KK), np.float32)               # [ci, layer, co, kk]
    for layer, w in ((0, w_add1), (1, w_add2)):
        w = np.asarray(w, np.float32)
        for kk in range(KK):
            kh, kw = divmod(kk, 3)
            wadd[:, layer, :, kk] = w[:, :, kh, kw].T        # [ci, co]
    # two one-hot families: columns [0,2c) select with value -2.0 (for DVE
    # tiles min(y-w,0)), columns [2c,4c) with value +2.0 (for ScalarE tiles
    # relu(w-y)); sum = 2*relu(w-y) contributions either way.
    onehot = np.zeros((c, 4 * c), ml_dtypes.bfloat16)
    onehot[:, c] = -2.0
    onehot[:, 3 * c] = 2.0
    gb = np.stack([np.asarray(v, np.float32) for v in
                   (bn1_gamma, bn1_beta, bn2_gamma, bn2_beta)], axis=1)
    return {"wq": np.ascontiguousarray(wq),
            "wadd": np.ascontiguousarray(wadd),
            "onehot": np.ascontiguousarray(onehot),
            "gb": np.ascontiguousarray(gb)}


def build_program(c: int, n_img: int, n_cores: int):
    nc = bacc.Bacc("TRN2", target_bir_lowering=False, debug=False,
                   num_devices=n_cores)
    x_t = nc.dram_tensor("x", [n_img, c, H, W], F32, kind="ExternalInput")
    wq_t = nc.dram_tensor("wq", [2, KK, c, c], F32, kind="ExternalInput")
    wadd_t = nc.dram_tensor("wadd", [c, 2, c, KK], F32, kind="ExternalInput")
    oh_t = nc.dram_tensor("onehot", [c, 4 * c], BF16, kind="ExternalInput")
    gb_t = nc.dram_tensor("gb", [c, 4], F32, kind="ExternalInput")
    out_t = nc.dram_tensor("out", [n_img, c, H, W], F32, kind="ExternalOutput")
    with tile.TileContext(nc) as tc:
        build_body(tc, out_t.ap(), x_t.ap(), wq_t.ap(), wadd_t.ap(),
                   oh_t.ap(), gb_t.ap(), c, n_img, n_cores)
    nc.compile()
    return nc


def run(inputs: dict, trace: bool = False):
    from concourse.bass_utils import run_bass_kernel_spmd
    x = np.ascontiguousarray(np.asarray(inputs["x"], np.float32))
    n, c = x.shape[0], x.shape[1]
    n_img = n // N_CORES
    host = prep_weights(inputs["w_shift1"], inputs["w_add1"],
                        inputs["w_shift2"], inputs["w_add2"],
                        inputs["bn1_gamma"], inputs["bn1_beta"],
                        inputs["bn2_gamma"], inputs["bn2_beta"], c)
    nc = build_program(c, n_img, N_CORES)
    in_maps = []
    for k in range(N_CORES):
        m = dict(host)
        m["x"] = np.ascontiguousarray(x[k * n_img:(k + 1) * n_img])
        in_maps.append(m)
    res = run_bass_kernel_spmd(nc, in_maps, core_ids=list(range(N_CORES)),
                               trace=trace)
    out = np.concatenate([r["out"] for r in res.results], axis=0)
    return out, res


def kernel(**inputs) -> np.ndarray:
    return run(inputs)[0]
